# revision 8
# baseline (speedup 1.0000x reference)
"""AnomalyAttention Trainium2 kernel — 8 NeuronCores, data-parallel over batch.

Computes, for B=32, L=512, H=8, E=64 (shapes hardcoded):
    scores   = Q @ K^T (causal masked), series = softmax(scores/8)
    V_out    = series @ V
    prior    = 1/(sqrt(2pi) sig) * exp(-(i-j)^2 / (2 sig^2))
    sigma_out= broadcast(sig)  with sig = 3^(sigmoid(5*sigma)+1e-5) - 1

Each of the 8 cores handles 4 batches; all heads computed locally, no
collectives. The host pre-transposes Q/K to [E, L] layout and casts Q/K/V to
bf16 (TensorEngine compute dtype), and appends a ones-column per head to V so
the series@V matmul also produces the softmax row-sums. On-chip: QK^T and
series@V on the TensorEngine (fp32 accumulation), exp/prior/sigma-broadcast on
the ScalarEngine, normalization and masking on the VectorEngine. The causal
structure skips all fully-masked score blocks; their series output is zero.
"""
import math
import sys
import types
from contextlib import ExitStack

sys.path.insert(0, "/opt/trn_rl_repo")

import numpy as np

# NTFF profile hook shim: the container's antenv package lacks axon_hooks, so
# register an equivalent module before concourse imports it (trace=True path).
if "antenv.axon_hooks" not in sys.modules:
    _hook_mod = types.ModuleType("antenv.axon_hooks")
    _hook_store = [None]
    _hook_mod.set_axon_ntff_profile_hook = lambda h: _hook_store.__setitem__(0, h)
    _hook_mod.get_axon_ntff_profile_hook = lambda: _hook_store[0]
    sys.modules["antenv.axon_hooks"] = _hook_mod
    try:
        import antenv

        antenv.axon_hooks = _hook_mod
        from trn_agent_boot.trn_boot import _ntff_profile_via_ctypes

        _hook = _ntff_profile_via_ctypes("/opt/axon/libaxon_pjrt.so")
        if _hook is not None:
            _hook_mod.set_axon_ntff_profile_hook(_hook)
    except Exception:
        pass

import ml_dtypes
import concourse.bass as bass
import concourse.tile as tile
from concourse import mybir
from concourse.bass_utils import run_bass_kernel_spmd
from concourse.masks import make_identity

F32 = mybir.dt.float32
BF16 = mybir.dt.bfloat16
ACT = mybir.ActivationFunctionType

N_CORES = 8
B, L, H, E = 32, 512, 8, 64
BL = B // N_CORES  # batches per core
P = 128
NB = L // P  # 4 row blocks
HP = H // 2  # head pairs (two heads' E dims stacked on 128 partitions)
E1 = E + 1  # V columns per head incl. the ones column
LN3 = math.log(3.0)
NEG_HALF_LN_2PI = -0.5 * math.log(2.0 * math.pi)


def _split_excess_waits(nc):
    """This container's walrus accepts at most one sync-wait per instruction
    (two for EventSemaphore), but Tile attaches one wait per dependency.
    Hoist excess waits onto dedicated same-engine NOPs placed immediately
    before the instruction — equivalent for monotone (sem-ge) waits."""
    fixn = [0]
    for f in nc.m.functions:
        for bb in f.blocks:
            out = []
            changed = False
            for inst in bb.instructions:
                si = inst.sync_info
                n = len(si.on_wait) if si and si.on_wait else 0
                cap = 2 if isinstance(inst, mybir.InstEventSemaphore) else 1
                if n > cap:
                    waits = list(si.on_wait)
                    # keep non-monotone (eq) waits on the instruction itself
                    waits.sort(key=lambda w: "ge" in w.wait_mode)
                    keep, hoist = waits[:cap], waits[cap:]
                    for w in hoist:
                        assert "ge" in w.wait_mode, w
                        fixn[0] += 1
                        nop = mybir.InstNoOp(
                            name=f"Iwfix-{fixn[0]}",
                            engine=inst.engine,
                            ins=[],
                            outs=[],
                            bass_nofuse=True,
                        )
                        nop.sync_info = type(si)(on_wait=[w], on_update=[])
                        out.append(nop)
                    si.on_wait = keep
                    changed = True
                out.append(inst)
            if changed:
                bb.instructions = out
    return nc


def _build():
    nc = bass.Bass("TRN2")
    qt = nc.declare_dram_parameter("qt", [BL, HP, P, L], BF16, isOutput=False)
    kt = nc.declare_dram_parameter("kt", [BL, HP, P, L], BF16, isOutput=False)
    v5 = nc.declare_dram_parameter("v5", [BL, L, H * E1], BF16, isOutput=False)
    sg = nc.declare_dram_parameter("sg", [BL, L, H], F32, isOutput=False)
    d2 = nc.declare_dram_parameter("d2", [L, L], F32, isOutput=False)
    vo = nc.declare_dram_parameter("vo", [BL, L, H * E], F32, isOutput=True)
    so = nc.declare_dram_parameter("so", [BL, H, L, L], F32, isOutput=True)
    po = nc.declare_dram_parameter("po", [BL, H, L, L], F32, isOutput=True)
    go = nc.declare_dram_parameter("go", [BL, H, L, L], F32, isOutput=True)

    with ExitStack() as ctx:
        tc = ctx.enter_context(tile.TileContext(nc))
        consts = ctx.enter_context(tc.tile_pool(name="consts", bufs=1))
        slabs = ctx.enter_context(tc.tile_pool(name="slabs", bufs=2))
        work = ctx.enter_context(tc.tile_pool(name="work", bufs=4))
        eTp = ctx.enter_context(tc.tile_pool(name="eTp", bufs=8))
        small = ctx.enter_context(tc.tile_pool(name="small", bufs=12))
        ps_sc = ctx.enter_context(tc.tile_pool(name="ps_sc", bufs=3, space="PSUM"))
        ps_tr = ctx.enter_context(tc.tile_pool(name="ps_tr", bufs=3, space="PSUM"))
        ps_av = ctx.enter_context(tc.tile_pool(name="ps_av", bufs=2, space="PSUM"))

        ident = consts.tile([P, P], BF16)
        make_identity(nc, ident)
        # multiplicative causal mask for the diagonal block: 1 on j<=i, else 0
        trimask = consts.tile([P, P], BF16)
        nc.gpsimd.memset(trimask, 1.0)
        nc.gpsimd.affine_select(
            out=trimask,
            in_=trimask,
            compare_op=mybir.AluOpType.is_ge,
            fill=0.0,
            base=0,
            pattern=[[-1, P]],
            channel_multiplier=1,
        )
        zerot = consts.tile([P, L - P], F32)
        nc.vector.memset(zerot, 0.0)
        bias_ln3eps = consts.tile([P, 1], F32)
        nc.vector.memset(bias_ln3eps, 1e-5 * LN3)
        d2t = []
        for i in range(NB):
            t = consts.tile([P, L], F32, tag=f"d2_{i}")
            nc.sync.dma_start(out=t, in_=d2[i * P : (i + 1) * P, :])
            d2t.append(t)

        for b in range(BL):
            # ---- load pre-transposed Q/K and ones-augmented V (bf16) ----
            qT, kT = [], []
            for hp in range(HP):
                qTt = slabs.tile([P, L], BF16, tag=f"qT{hp}")
                nc.sync.dma_start(out=qTt, in_=qt[b, hp])
                qT.append(qTt)
                kTt = slabs.tile([P, L], BF16, tag=f"kT{hp}")
                nc.sync.dma_start(out=kTt, in_=kt[b, hp])
                kT.append(kTt)
            vb = []
            for t in range(NB):
                vt = slabs.tile([P, H * E1], BF16, tag=f"v{t}")
                nc.sync.dma_start(out=vt, in_=v5[b, t * P : (t + 1) * P, :])
                vb.append(vt)
            # ---- sigma chain per L-block: sig, -1/(2 sig^2), ln(c) ----
            sig_t, ns_t, lc_t = [], [], []
            for t in range(NB):
                sraw = small.tile([P, H], F32, tag="sraw")
                nc.sync.dma_start(out=sraw, in_=sg[b, t * P : (t + 1) * P, :])
                s1 = small.tile([P, H], F32, tag="s1")
                nc.scalar.activation(out=s1, in_=sraw, func=ACT.Sigmoid, scale=5.0)
                u = small.tile([P, H], F32, tag="u")
                nc.scalar.activation(
                    out=u, in_=s1, func=ACT.Exp, scale=LN3, bias=bias_ln3eps
                )
                sig = slabs.tile([P, H], F32, tag=f"sig{t}")
                nc.vector.tensor_scalar_add(out=sig, in0=u, scalar1=-1.0)
                sq = small.tile([P, H], F32, tag="sq")
                nc.vector.tensor_mul(out=sq, in0=sig, in1=sig)
                rq = small.tile([P, H], F32, tag="rq")
                nc.vector.reciprocal(out=rq, in_=sq)
                ns = slabs.tile([P, H], F32, tag=f"ns{t}")
                nc.vector.tensor_scalar_mul(out=ns, in0=rq, scalar1=-0.5)
                ln = small.tile([P, H], F32, tag="ln")
                nc.scalar.activation(out=ln, in_=sig, func=ACT.Ln)
                lc = slabs.tile([P, H], F32, tag=f"lc{t}")
                nc.vector.tensor_scalar(
                    out=lc,
                    in0=ln,
                    scalar1=-1.0,
                    scalar2=NEG_HALF_LN_2PI,
                    op0=mybir.AluOpType.mult,
                    op1=mybir.AluOpType.add,
                )
                sig_t.append(sig)
                ns_t.append(ns)
                lc_t.append(lc)

            # ---- per (head, row-block) attention + prior ----
            for h in range(H):
                hp, ho = h // 2, (h % 2) * E
                for i in range(NB):
                    W = (i + 1) * P  # causal width of this row block
                    rows = slice(i * P, (i + 1) * P)
                    sc = ps_sc.tile([P, L], F32, tag="sc")
                    nc.tensor.matmul(
                        sc[:, :W],
                        lhsT=qT[hp][ho : ho + E, rows],
                        rhs=kT[hp][ho : ho + E, :W],
                        start=True,
                        stop=True,
                    )
                    expb = work.tile([P, L], BF16, tag="exp")
                    nc.scalar.activation(
                        out=expb[:, :W], in_=sc[:, :W], func=ACT.Exp, scale=0.125
                    )
                    # zero the strictly-upper triangle of the diagonal block
                    nc.vector.tensor_mul(
                        out=expb[:, i * P : W],
                        in0=expb[:, i * P : W],
                        in1=trimask,
                    )
                    # series @ [V | 1]: transposed exp blocks; col E is rowsum
                    eTs = []
                    for j in range(i + 1):
                        pt = ps_tr.tile([P, P], BF16, tag="ps_tr")
                        nc.tensor.transpose(pt, expb[:, j * P : (j + 1) * P], ident)
                        eT = eTp.tile([P, P], BF16, tag="eT")
                        nc.vector.tensor_copy(out=eT, in_=pt)
                        eTs.append(eT)
                    va = ps_av.tile([P, E1], F32, tag="va")
                    for j in range(i + 1):
                        nc.tensor.matmul(
                            va,
                            lhsT=eTs[j],
                            rhs=vb[j][:, h * E1 : (h + 1) * E1],
                            start=(j == 0),
                            stop=(j == i),
                        )
                    rinv = small.tile([P, 1], F32, tag="rinv")
                    nc.vector.reciprocal(out=rinv, in_=va[:, E : E + 1])
                    serf = work.tile([P, L], F32, tag="ser")
                    nc.vector.tensor_scalar_mul(
                        out=serf[:, :W], in0=expb[:, :W], scalar1=rinv
                    )
                    nc.sync.dma_start(out=so[b, h, rows, :W], in_=serf[:, :W])
                    if W < L:
                        nc.sync.dma_start(
                            out=so[b, h, rows, W:], in_=zerot[:, : L - W]
                        )
                    vos = work.tile([P, E], F32, tag="vo")
                    nc.vector.tensor_scalar_mul(
                        out=vos, in0=va[:, :E], scalar1=rinv
                    )
                    nc.sync.dma_start(
                        out=vo[b, rows, h * E : (h + 1) * E], in_=vos
                    )
                    # prior: exp(d2 * (-1/(2 sig^2)) + ln c) in one ACT pass
                    pri = work.tile([P, L], F32, tag="pri")
                    nc.scalar.activation(
                        out=pri,
                        in_=d2t[i],
                        func=ACT.Exp,
                        scale=ns_t[i][:, h : h + 1],
                        bias=lc_t[i][:, h : h + 1],
                    )
                    nc.sync.dma_start(out=po[b, h, rows, :], in_=pri)
                    # sigma_out: broadcast sig along the row (ACT copy)
                    sgo = work.tile([P, L], F32, tag="sgo")
                    nc.scalar.activation(
                        out=sgo,
                        in_=sig_t[i][:, h : h + 1].to_broadcast([P, L]),
                        func=ACT.Copy,
                    )
                    nc.sync.dma_start(out=go[b, h, rows, :], in_=sgo)
    return _split_excess_waits(nc)


_nc_cache = None
last_results = None


def kernel(queries, keys, values, sigma, attention_mask=None, **_unused):
    """Full-input entry point: shard over 8 cores, run, gather."""
    global _nc_cache, last_results
    if _nc_cache is None:
        _nc_cache = _build()
    nc = _nc_cache

    queries = np.ascontiguousarray(np.asarray(queries), dtype=np.float32)
    keys = np.ascontiguousarray(np.asarray(keys), dtype=np.float32)
    values = np.ascontiguousarray(np.asarray(values), dtype=np.float32)
    sigma = np.ascontiguousarray(np.asarray(sigma), dtype=np.float32)

    bf = ml_dtypes.bfloat16
    # Q/K transposed to [B, head-pair, 2E, L] so two heads' E dims stack on
    # the 128 SBUF partitions; V gets a ones column per head (row-sum trick).
    qT = np.ascontiguousarray(
        queries.reshape(B, L, HP, 2 * E).transpose(0, 2, 3, 1)
    ).astype(bf)
    kT = np.ascontiguousarray(
        keys.reshape(B, L, HP, 2 * E).transpose(0, 2, 3, 1)
    ).astype(bf)
    v5 = np.ones((B, L, H, E1), dtype=bf)
    v5[..., :E] = values.reshape(B, L, H, E).astype(bf)
    v5 = v5.reshape(B, L, H * E1)

    idx = np.arange(L, dtype=np.float32)
    d2 = (idx[:, None] - idx[None, :]) ** 2

    in_maps = []
    for c in range(N_CORES):
        bs = slice(c * BL, (c + 1) * BL)
        in_maps.append(
            {
                "qt": qT[bs],
                "kt": kT[bs],
                "v5": v5[bs],
                "sg": sigma[bs],
                "d2": d2,
            }
        )

    res = run_bass_kernel_spmd(nc, in_maps, core_ids=list(range(N_CORES)))
    last_results = res

    V = np.concatenate(
        [res.results[c]["vo"].reshape(BL, L, H, E) for c in range(N_CORES)], axis=0
    )
    series = np.concatenate([res.results[c]["so"] for c in range(N_CORES)], axis=0)
    prior = np.concatenate([res.results[c]["po"] for c in range(N_CORES)], axis=0)
    sigma_out = np.concatenate([res.results[c]["go"] for c in range(N_CORES)], axis=0)
    return V, series, prior, sigma_out


# revision 10
# speedup vs baseline: 1.6018x; 1.6018x over previous
"""AnomalyAttention Trainium2 kernel — 8 NeuronCores, data-parallel over batch.

Computes, for B=32, L=512, H=8, E=64 (shapes hardcoded):
    scores   = Q @ K^T (causal masked), series = softmax(scores/8)
    V_out    = series @ V
    prior    = 1/(sqrt(2pi) sig) * exp(-(i-j)^2 / (2 sig^2))
    sigma_out= broadcast(sig)  with sig = 3^(sigmoid(5*sigma)+1e-5) - 1

Each of the 8 cores handles 4 batches; all heads computed locally, no
collectives. The host pre-transposes Q/K to [E, L] layout and casts Q/K/V to
bf16 (TensorEngine compute dtype), and appends a ones-column per head to V so
the series@V matmul also produces the softmax row-sums. On-chip: QK^T and
series@V on the TensorEngine (fp32 accumulation), exp/prior/sigma-broadcast on
the ScalarEngine, normalization and masking on the VectorEngine. The causal
structure skips all fully-masked score blocks; their series output is zero.
"""
import math
import sys
import types
from contextlib import ExitStack

sys.path.insert(0, "/opt/trn_rl_repo")

import numpy as np

# NTFF profile hook shim: the container's antenv package lacks axon_hooks, so
# register an equivalent module before concourse imports it (trace=True path).
if "antenv.axon_hooks" not in sys.modules:
    _hook_mod = types.ModuleType("antenv.axon_hooks")
    _hook_store = [None]
    _hook_mod.set_axon_ntff_profile_hook = lambda h: _hook_store.__setitem__(0, h)
    _hook_mod.get_axon_ntff_profile_hook = lambda: _hook_store[0]
    sys.modules["antenv.axon_hooks"] = _hook_mod
    try:
        import antenv

        antenv.axon_hooks = _hook_mod
        from trn_agent_boot.trn_boot import _ntff_profile_via_ctypes

        _hook = _ntff_profile_via_ctypes("/opt/axon/libaxon_pjrt.so")
        if _hook is not None:
            _hook_mod.set_axon_ntff_profile_hook(_hook)
    except Exception:
        pass

import ml_dtypes
import concourse.bass as bass
import concourse.tile as tile
from concourse import mybir
from concourse.bass_utils import run_bass_kernel_spmd
from concourse.masks import make_identity

F32 = mybir.dt.float32
BF16 = mybir.dt.bfloat16
ACT = mybir.ActivationFunctionType

N_CORES = 8
B, L, H, E = 32, 512, 8, 64
BL = B // N_CORES  # batches per core
P = 128
NB = L // P  # 4 row blocks
HP = H // 2  # head pairs (two heads' E dims stacked on 128 partitions)
E1 = E + 1  # V columns per head incl. the ones column
LN3 = math.log(3.0)
NEG_HALF_LN_2PI = -0.5 * math.log(2.0 * math.pi)


def _split_excess_waits(nc):
    """This container's walrus accepts at most one sync-wait per instruction
    (two for EventSemaphore), but Tile attaches one wait per dependency.
    Hoist excess waits onto dedicated same-engine NOPs placed immediately
    before the instruction — equivalent for monotone (sem-ge) waits."""
    fixn = [0]
    for f in nc.m.functions:
        for bb in f.blocks:
            out = []
            changed = False
            for inst in bb.instructions:
                si = inst.sync_info
                n = len(si.on_wait) if si and si.on_wait else 0
                cap = 2 if isinstance(inst, mybir.InstEventSemaphore) else 1
                if n > cap:
                    waits = list(si.on_wait)
                    # keep non-monotone (eq) waits on the instruction itself
                    waits.sort(key=lambda w: "ge" in w.wait_mode)
                    keep, hoist = waits[:cap], waits[cap:]
                    for w in hoist:
                        assert "ge" in w.wait_mode, w
                        fixn[0] += 1
                        nop = mybir.InstNoOp(
                            name=f"Iwfix-{fixn[0]}",
                            engine=inst.engine,
                            ins=[],
                            outs=[],
                            bass_nofuse=True,
                        )
                        nop.sync_info = type(si)(on_wait=[w], on_update=[])
                        out.append(nop)
                    si.on_wait = keep
                    changed = True
                out.append(inst)
            if changed:
                bb.instructions = out
    return nc


def _build():
    nc = bass.Bass("TRN2")
    qt = nc.declare_dram_parameter("qt", [BL, HP, P, L], BF16, isOutput=False)
    kt = nc.declare_dram_parameter("kt", [BL, HP, P, L], BF16, isOutput=False)
    v5 = nc.declare_dram_parameter("v5", [BL, L, H * E1], BF16, isOutput=False)
    sg = nc.declare_dram_parameter("sg", [BL, L, H], F32, isOutput=False)
    d2 = nc.declare_dram_parameter("d2", [L, L], F32, isOutput=False)
    vo = nc.declare_dram_parameter("vo", [BL, L, H * E], F32, isOutput=True)
    so = nc.declare_dram_parameter("so", [BL, H, L, L], F32, isOutput=True)
    po = nc.declare_dram_parameter("po", [BL, H, L, L], F32, isOutput=True)
    go = nc.declare_dram_parameter("go", [BL, H, L, L], F32, isOutput=True)

    with ExitStack() as ctx:
        tc = ctx.enter_context(tile.TileContext(nc))
        consts = ctx.enter_context(tc.tile_pool(name="consts", bufs=1))
        slabs = ctx.enter_context(tc.tile_pool(name="slabs", bufs=2))
        work = ctx.enter_context(tc.tile_pool(name="work", bufs=4))
        eTp = ctx.enter_context(tc.tile_pool(name="eTp", bufs=8))
        small = ctx.enter_context(tc.tile_pool(name="small", bufs=12))
        ps_sc = ctx.enter_context(tc.tile_pool(name="ps_sc", bufs=3, space="PSUM"))
        ps_tr = ctx.enter_context(tc.tile_pool(name="ps_tr", bufs=3, space="PSUM"))
        ps_av = ctx.enter_context(tc.tile_pool(name="ps_av", bufs=2, space="PSUM"))

        ident = consts.tile([P, P], BF16)
        make_identity(nc, ident)
        # multiplicative causal mask for the diagonal block: 1 on j<=i, else 0
        trimask = consts.tile([P, P], BF16)
        nc.gpsimd.memset(trimask, 1.0)
        nc.gpsimd.affine_select(
            out=trimask,
            in_=trimask,
            compare_op=mybir.AluOpType.is_ge,
            fill=0.0,
            base=0,
            pattern=[[-1, P]],
            channel_multiplier=1,
        )
        bias_ln3eps = consts.tile([P, 1], F32)
        nc.vector.memset(bias_ln3eps, 1e-5 * LN3)
        d2t = []
        for i in range(NB):
            t = consts.tile([P, L], F32, tag=f"d2_{i}")
            nc.sync.dma_start(out=t, in_=d2[i * P : (i + 1) * P, :])
            d2t.append(t)

        for b in range(BL):
            # ---- load pre-transposed Q/K and ones-augmented V (bf16) ----
            qT, kT = [], []
            for hp in range(HP):
                qTt = slabs.tile([P, L], BF16, tag=f"qT{hp}")
                nc.sync.dma_start(out=qTt, in_=qt[b, hp])
                qT.append(qTt)
                kTt = slabs.tile([P, L], BF16, tag=f"kT{hp}")
                nc.sync.dma_start(out=kTt, in_=kt[b, hp])
                kT.append(kTt)
            vb = []
            for t in range(NB):
                vt = slabs.tile([P, H * E1], BF16, tag=f"v{t}")
                nc.sync.dma_start(out=vt, in_=v5[b, t * P : (t + 1) * P, :])
                vb.append(vt)
            # ---- sigma chain per L-block: sig, -1/(2 sig^2), ln(c) ----
            sig_t, ns_t, lc_t = [], [], []
            for t in range(NB):
                sraw = small.tile([P, H], F32, tag="sraw")
                nc.sync.dma_start(out=sraw, in_=sg[b, t * P : (t + 1) * P, :])
                s1 = small.tile([P, H], F32, tag="s1")
                nc.scalar.activation(out=s1, in_=sraw, func=ACT.Sigmoid, scale=5.0)
                u = small.tile([P, H], F32, tag="u")
                nc.scalar.activation(
                    out=u, in_=s1, func=ACT.Exp, scale=LN3, bias=bias_ln3eps
                )
                sig = slabs.tile([P, H], F32, tag=f"sig{t}")
                nc.vector.tensor_scalar_add(out=sig, in0=u, scalar1=-1.0)
                sq = small.tile([P, H], F32, tag="sq")
                nc.vector.tensor_mul(out=sq, in0=sig, in1=sig)
                rq = small.tile([P, H], F32, tag="rq")
                nc.vector.reciprocal(out=rq, in_=sq)
                ns = slabs.tile([P, H], F32, tag=f"ns{t}")
                nc.vector.tensor_scalar_mul(out=ns, in0=rq, scalar1=-0.5)
                ln = small.tile([P, H], F32, tag="ln")
                nc.scalar.activation(out=ln, in_=sig, func=ACT.Ln)
                lc = slabs.tile([P, H], F32, tag=f"lc{t}")
                nc.vector.tensor_scalar(
                    out=lc,
                    in0=ln,
                    scalar1=-1.0,
                    scalar2=NEG_HALF_LN_2PI,
                    op0=mybir.AluOpType.mult,
                    op1=mybir.AluOpType.add,
                )
                sig_t.append(sig)
                ns_t.append(ns)
                lc_t.append(lc)

            # ---- per (row-block, head) attention + prior ----
            # Note: series rows beyond the causal width are never written —
            # the runner donates pre-zeroed output buffers, so they stay 0.
            for i in range(NB):
                W = (i + 1) * P  # causal width of this row block
                rows = slice(i * P, (i + 1) * P)
                vos_all = work.tile([P, H * E], F32, tag="voall")
                for h in range(H):
                    hp, ho = h // 2, (h % 2) * E
                    sc = ps_sc.tile([P, L], F32, tag="sc")
                    nc.tensor.matmul(
                        sc[:, :W],
                        lhsT=qT[hp][ho : ho + E, rows],
                        rhs=kT[hp][ho : ho + E, :W],
                        start=True,
                        stop=True,
                    )
                    expb = work.tile([P, L], BF16, tag="exp")
                    nc.scalar.activation(
                        out=expb[:, :W], in_=sc[:, :W], func=ACT.Exp, scale=0.125
                    )
                    # zero the strictly-upper triangle of the diagonal block
                    nc.vector.tensor_mul(
                        out=expb[:, i * P : W],
                        in0=expb[:, i * P : W],
                        in1=trimask,
                    )
                    # series @ [V | 1]: transposed exp blocks; col E is rowsum
                    eTs = []
                    for j in range(i + 1):
                        pt = ps_tr.tile([P, P], BF16, tag="ps_tr")
                        nc.tensor.transpose(pt, expb[:, j * P : (j + 1) * P], ident)
                        eT = eTp.tile([P, P], BF16, tag="eT")
                        nc.vector.tensor_copy(out=eT, in_=pt)
                        eTs.append(eT)
                    va = ps_av.tile([P, E1], F32, tag="va")
                    for j in range(i + 1):
                        nc.tensor.matmul(
                            va,
                            lhsT=eTs[j],
                            rhs=vb[j][:, h * E1 : (h + 1) * E1],
                            start=(j == 0),
                            stop=(j == i),
                        )
                    rinv = small.tile([P, 1], F32, tag="rinv")
                    nc.vector.reciprocal(out=rinv, in_=va[:, E : E + 1])
                    serf = work.tile([P, L], F32, tag="ser")
                    nc.vector.tensor_scalar_mul(
                        out=serf[:, :W], in0=expb[:, :W], scalar1=rinv
                    )
                    nc.sync.dma_start(out=so[b, h, rows, :W], in_=serf[:, :W])
                    nc.vector.tensor_scalar_mul(
                        out=vos_all[:, h * E : (h + 1) * E],
                        in0=va[:, :E],
                        scalar1=rinv,
                    )
                    # prior: exp(d2 * (-1/(2 sig^2)) + ln c) in one ACT pass
                    pri = work.tile([P, L], F32, tag="pri")
                    nc.scalar.activation(
                        out=pri,
                        in_=d2t[i],
                        func=ACT.Exp,
                        scale=ns_t[i][:, h : h + 1],
                        bias=lc_t[i][:, h : h + 1],
                    )
                    nc.scalar.dma_start(out=po[b, h, rows, :], in_=pri)
                    # sigma_out: broadcast sig along the row; alternate the
                    # producing engine and the issuing DMA queue per head
                    sgo = work.tile([P, L], F32, tag="sgo")
                    sig_ap = sig_t[i][:, h : h + 1].to_broadcast([P, L])
                    if h % 2 == 0:
                        nc.scalar.activation(out=sgo, in_=sig_ap, func=ACT.Copy)
                        nc.sync.dma_start(out=go[b, h, rows, :], in_=sgo)
                    else:
                        nc.vector.tensor_copy(out=sgo, in_=sig_ap)
                        nc.scalar.dma_start(out=go[b, h, rows, :], in_=sgo)
                nc.sync.dma_start(out=vo[b, rows, :], in_=vos_all)
    return _split_excess_waits(nc)


_nc_cache = None
last_results = None


def kernel(queries, keys, values, sigma, attention_mask=None, **_unused):
    """Full-input entry point: shard over 8 cores, run, gather."""
    global _nc_cache, last_results
    if _nc_cache is None:
        _nc_cache = _build()
    nc = _nc_cache

    queries = np.ascontiguousarray(np.asarray(queries), dtype=np.float32)
    keys = np.ascontiguousarray(np.asarray(keys), dtype=np.float32)
    values = np.ascontiguousarray(np.asarray(values), dtype=np.float32)
    sigma = np.ascontiguousarray(np.asarray(sigma), dtype=np.float32)

    bf = ml_dtypes.bfloat16
    # Q/K transposed to [B, head-pair, 2E, L] so two heads' E dims stack on
    # the 128 SBUF partitions; V gets a ones column per head (row-sum trick).
    qT = np.ascontiguousarray(
        queries.reshape(B, L, HP, 2 * E).transpose(0, 2, 3, 1)
    ).astype(bf)
    kT = np.ascontiguousarray(
        keys.reshape(B, L, HP, 2 * E).transpose(0, 2, 3, 1)
    ).astype(bf)
    v5 = np.ones((B, L, H, E1), dtype=bf)
    v5[..., :E] = values.reshape(B, L, H, E).astype(bf)
    v5 = v5.reshape(B, L, H * E1)

    idx = np.arange(L, dtype=np.float32)
    d2 = (idx[:, None] - idx[None, :]) ** 2

    in_maps = []
    for c in range(N_CORES):
        bs = slice(c * BL, (c + 1) * BL)
        in_maps.append(
            {
                "qt": qT[bs],
                "kt": kT[bs],
                "v5": v5[bs],
                "sg": sigma[bs],
                "d2": d2,
            }
        )

    res = run_bass_kernel_spmd(nc, in_maps, core_ids=list(range(N_CORES)))
    last_results = res

    V = np.concatenate(
        [res.results[c]["vo"].reshape(BL, L, H, E) for c in range(N_CORES)], axis=0
    )
    series = np.concatenate([res.results[c]["so"] for c in range(N_CORES)], axis=0)
    prior = np.concatenate([res.results[c]["po"] for c in range(N_CORES)], axis=0)
    sigma_out = np.concatenate([res.results[c]["go"] for c in range(N_CORES)], axis=0)
    return V, series, prior, sigma_out
